# revision 23
# baseline (speedup 1.0000x reference)
"""Trainium2 Bass kernel for CustomCosineSimCodebook (vq_codebook).

Problem: x [8, 2048, 512] f32 tokens, embed [1, 8192, 512] f32 unit-norm
codebook. Outputs (matching the reference nn.Module):
  quantize [8, 2048, 512] f32  -- codebook rows gathered at argmax indices
  ind      [8, 2048]      int32 -- argmax over codes of x @ embed^T
  dist     [1, 8, 2048, 8192] f32 -- full similarity scores

Sharding: data-parallel over the b*n token axis; core k takes batch row k
(2048 tokens). The codebook is replicated.

Numerics: the scores must be fp32-grade (the argmax feeds an exact gather;
one flipped index is a large error). fp32 matmul on the PE costs 4x. Instead
each operand is split hi/lo into two fp16 halves (lo parts pre-scaled by
2^11 so they stay in fp16 normal range, and x_hi additionally provided
pre-scaled by 2^11 as xh_s), and
    2^11 * dist = xh_s @ e_hi + x_lo' @ e_hi + x_hi @ e_lo'
runs as 3 fp16 passes at full PE rate, all accumulating into ONE fp32 PSUM
bank at 2^11 scale; a single scalar-engine Copy with scale=2^-11 (exact)
writes each chunk to SBUF. Dropped lo*lo term ~2^-23 relative => score
error ~1e-6: fp32-grade.

Argmax: DVE MAX/MAX_INDEX over each token's full 8192-score row in SBUF
(first-match on ties, same as jnp.argmax). Quantize: indirect-DMA gather of
fp32 codebook rows (bit-exact vs reference given equal indices).
"""

import numpy as np

import concourse.bass as bass
import concourse.mybir as mybir
import concourse.tile as tile

B, N, D, C = 8, 2048, 512, 8192
NCORES = 8
TOK = B * N // NCORES        # 2048 tokens per core
P = 128                      # partitions
KC = D // P                  # 4 contraction chunks
NCH = C // 512               # 16 n-chunks of 512 codes
GROUP = 8                    # n-chunks in flight (one PSUM bank each)
SCALE = 2.0 ** 11
F16_MIN_NORMAL = 6.104e-5

LAST_RESULT = None           # BassKernelResults of the most recent run


def build(n_mtiles=TOK // P, repeats=1):
    """Build the per-core Bass module (SPMD: same NEFF on all cores).
    repeats>1 re-runs the whole compute body (idempotent outputs) so test
    harnesses can isolate HW exec time by differencing wall times."""
    nc = bass.Bass()
    tok = n_mtiles * P

    xh_d = nc.dram_tensor("xh", [D, tok], mybir.dt.float16, kind="ExternalInput")
    xs_d = nc.dram_tensor("xs", [D, tok], mybir.dt.float16, kind="ExternalInput")
    xl_d = nc.dram_tensor("xl", [D, tok], mybir.dt.float16, kind="ExternalInput")
    eh_d = nc.dram_tensor("eh", [D, C], mybir.dt.float16, kind="ExternalInput")
    el_d = nc.dram_tensor("el", [D, C], mybir.dt.float16, kind="ExternalInput")
    emb_d = nc.dram_tensor("emb", [C, D], mybir.dt.float32, kind="ExternalInput")

    dist_d = nc.dram_tensor("dist", [tok, C], mybir.dt.float32, kind="ExternalOutput")
    ind_d = nc.dram_tensor("ind", [tok, 1], mybir.dt.uint32, kind="ExternalOutput")
    quant_d = nc.dram_tensor("quant", [tok, D], mybir.dt.float32, kind="ExternalOutput")

    with tile.TileContext(nc) as tc:
        with (
            tc.tile_pool(name="codebook", bufs=1) as cpool,
            tc.tile_pool(name="xtiles", bufs=2) as xpool,
            tc.tile_pool(name="distrow", bufs=1) as dpool,
            tc.tile_pool(name="outs", bufs=2) as opool,
            tc.tile_pool(name="psum", bufs=1, space="PSUM") as ppool,
        ):
            def load_x(m):
                ms_ = slice(m * P, (m + 1) * P)
                out = []
                for src, tag in ((xh_d, "xh"), (xs_d, "xs"), (xl_d, "xl")):
                    row = []
                    for kc in range(KC):
                        rs = slice(kc * P, (kc + 1) * P)
                        t = xpool.tile([P, P], mybir.dt.float16,
                                       tag=f"{tag}{kc}", name=f"{tag}{m}_{kc}")
                        nc.sync.dma_start(t[:], src[rs, ms_])
                        row.append(t)
                    out.append(row)
                return out

            # m=0's x tiles load first so the first matmul isn't queued
            # behind the 32 MB codebook stream.
            x0_tiles = load_x(0)

            # Codebook resident in SBUF, 128 KB/part total, split into
            # per-2048-column tiles so the first matmuls only wait for their
            # own column range instead of the whole 32 MB preload. DMAs are
            # emitted in consumption order (column-major, hi before lo).
            CCH = 2048
            eh_t = [[None] * (C // CCH) for _ in range(KC)]
            el_t = [[None] * (C // CCH) for _ in range(KC)]
            for j in range(0, C, CCH):
                cs = slice(j, j + CCH)
                for part, tiles in (("h", eh_t), ("l", el_t)):
                    src = eh_d if part == "h" else el_d
                    for kc in range(KC):
                        rs = slice(kc * P, (kc + 1) * P)
                        t = cpool.tile([P, CCH], mybir.dt.float16,
                                       name=f"e{part}s{kc}_{j}")
                        nc.sync.dma_start(t[:], src[rs, cs])
                        tiles[kc][j // CCH] = t

            def ecol(tiles_row, i):
                # 512-col slice i of a [C]-wide logical row split in CCH tiles
                return tiles_row[(i * 512) // CCH][
                    :, (i * 512) % CCH:(i * 512) % CCH + 512]

            for rep, m in ((r, mm) for r in range(repeats)
                           for mm in range(n_mtiles)):
                ms = slice(m * P, (m + 1) * P)
                if rep == 0 and m == 0:
                    xh_t, xs_t, xl_t = x0_tiles
                else:
                    xh_t, xs_t, xl_t = load_x(m)

                dist_row = dpool.tile([P, C], mybir.dt.float32, tag="dist_row")
                max8h = [opool.tile([P, 8], mybir.dt.float32, tag=f"max8h{g}",
                                    name=f"max8h{rep}_{m}_{g}")
                         for g in range(NCH // GROUP)]
                idx8h = [opool.tile([P, 8], mybir.dt.uint32, tag=f"idx8h{g}",
                                    name=f"idx8h{rep}_{m}_{g}")
                         for g in range(NCH // GROUP)]

                for g in range(NCH // GROUP):
                    pp = [ppool.tile([P, 512], mybir.dt.float32, tag=f"pp{i}",
                                     name=f"pp{rep}_{m}_{g}_{i}")
                          for i in range(GROUP)]
                    csl = [slice((g * GROUP + i) * 512, (g * GROUP + i + 1) * 512)
                           for i in range(GROUP)]
                    gi = [g * GROUP + i for i in range(GROUP)]
                    # 3 fp16 passes, one PSUM bank per chunk, at 2^11 scale.
                    # Each weight load feeds GROUP matmuls.
                    for kc in range(KC):
                        for i in range(GROUP):
                            nc.tensor.matmul(pp[i][:], xs_t[kc][:],
                                             ecol(eh_t[kc], gi[i]),
                                             start=(kc == 0), stop=False)
                    for kc in range(KC):
                        for i in range(GROUP):
                            nc.tensor.matmul(pp[i][:], xl_t[kc][:],
                                             ecol(eh_t[kc], gi[i]),
                                             start=False, stop=False)
                    for kc in range(KC):
                        for i in range(GROUP):
                            nc.tensor.matmul(pp[i][:], xh_t[kc][:],
                                             ecol(el_t[kc], gi[i]),
                                             start=False, stop=(kc == KC - 1))
                    for i in range(GROUP):
                        # dist = psum * 2^-11 (exact power-of-2 scale)
                        nc.scalar.activation(dist_row[:, csl[i]], pp[i][:],
                                             mybir.ActivationFunctionType.Copy,
                                             scale=1.0 / SCALE)
                    # stream this group's columns out (2 MB DMA)
                    gs = slice(g * GROUP * 512, (g + 1) * GROUP * 512)
                    nc.sync.dma_start(dist_d[ms, gs], dist_row[:, gs])

                    # per-half argmax so the scan overlaps the next group's
                    # matmuls instead of sitting on the kernel tail
                    nc.vector.max(max8h[g][:], dist_row[:, gs])
                    nc.vector.max_index(idx8h[g][:], max8h[g][:],
                                        dist_row[:, gs])

                # Merge halves (all [P,1] fp32 ops; indices <= 8191 are exact
                # in fp32). Ties pick the first half, matching jnp.argmax.
                mask = opool.tile([P, 1], mybir.dt.float32, tag="mask")
                nc.vector.tensor_tensor(mask[:], max8h[0][:, :1],
                                        max8h[1][:, :1],
                                        mybir.AluOpType.is_ge)
                ia = opool.tile([P, 1], mybir.dt.float32, tag="ia")
                ib = opool.tile([P, 1], mybir.dt.float32, tag="ib")
                nc.vector.tensor_copy(ia[:], idx8h[0][:, :1])
                nc.vector.tensor_copy(ib[:], idx8h[1][:, :1])
                # idx = ib + HALF + mask * (ia - ib - HALF)
                half_c = float(GROUP * 512)
                diff = opool.tile([P, 1], mybir.dt.float32, tag="diff")
                nc.vector.tensor_tensor(diff[:], ia[:], ib[:],
                                        mybir.AluOpType.subtract)
                nc.vector.tensor_scalar_sub(diff[:], diff[:], half_c)
                nc.vector.tensor_tensor(diff[:], diff[:], mask[:],
                                        mybir.AluOpType.mult)
                nc.vector.tensor_tensor(diff[:], diff[:], ib[:],
                                        mybir.AluOpType.add)
                nc.vector.tensor_scalar_add(diff[:], diff[:], half_c)
                idx_u = opool.tile([P, 1], mybir.dt.uint32, tag="idx_u")
                nc.vector.tensor_copy(idx_u[:], diff[:])

                qt = opool.tile([P, D], mybir.dt.float32, tag="qt")
                nc.gpsimd.indirect_dma_start(
                    out=qt[:], out_offset=None, in_=emb_d[:, :],
                    in_offset=bass.IndirectOffsetOnAxis(ap=idx_u[:, :1], axis=0))
                nc.sync.dma_start(quant_d[ms, :], qt[:])
                nc.sync.dma_start(ind_d[ms, :], idx_u[:, :1])
    return nc


def fix_sync_waits(nc, cap=1):
    """Walrus (this container's version) rejects instructions whose ISA struct
    carries more sync-wait commands than it has slots (DMA/STT: one). Tile
    emits up to ~4. Hoist excess waits onto InstNoOp's inserted immediately
    before the offender on the same engine: the sequencer executes stream-
    order, so blocking on the nop enforces a superset of the original
    ordering. Safe because Tile waits only reference sems incremented by
    other procs / earlier stream positions."""
    skip = {"InstEventSemaphore", "InstISA", "InstCall",
            "InstUnconditionalBranch", "InstCompareAndBranch"}
    n_id = 0
    for fn in nc.m.functions:
        for blk in fn.blocks:
            il = blk.instructions
            i = 0
            while i < len(il):
                inst = il[i]
                tn = type(inst).__name__
                si = inst.sync_info
                if (tn not in skip and si is not None
                        and len(si.on_wait) > cap):
                    waits = list(si.on_wait)
                    excess, keep = waits[:-cap], waits[-cap:]
                    inst.sync_info = mybir.SyncInfo(
                        on_wait=keep, on_update=list(si.on_update))
                    for w in excess:
                        nop = mybir.InstNoOp(
                            name=f"waitnop-{n_id}", engine=inst.engine,
                            ins=[], outs=[])
                        n_id += 1
                        nop.sync_info = mybir.SyncInfo(on_wait=[w], on_update=[])
                        il.insert(i, nop)
                        i += 1
                i += 1
    return nc


def split_fp16(a):
    """a (f32) -> (hi, lo) fp16 with lo pre-scaled by 2^11; no denormals.
    a ~= hi + lo * 2^-11 to ~2^-23 relative."""
    hi = a.astype(np.float16)
    hi = np.where(np.abs(hi) < F16_MIN_NORMAL, np.float16(0), hi)
    lo = ((a - hi.astype(np.float32)) * np.float32(SCALE)).astype(np.float16)
    lo = np.where(np.abs(lo) < F16_MIN_NORMAL, np.float16(0), lo)
    return hi, lo


_nc_cache = {}


def _get_nc(n_mtiles):
    if n_mtiles not in _nc_cache:
        _nc_cache[n_mtiles] = fix_sync_waits(build(n_mtiles))
    return _nc_cache[n_mtiles]


def kernel(x, embed, _trace=False):
    global LAST_RESULT
    from concourse.bass_utils import run_bass_kernel_spmd

    x = np.ascontiguousarray(np.asarray(x, dtype=np.float32))
    e = np.ascontiguousarray(np.asarray(embed, dtype=np.float32)[0])  # [C, D]

    eT = np.ascontiguousarray(e.T)                  # [D, C]
    eh, el = split_fp16(eT)
    in_maps = []
    for k in range(NCORES):
        xT = np.ascontiguousarray(x[k].T)           # [D, 2048]
        xh, xl = split_fp16(xT)
        xs = xh * np.float16(SCALE)                 # exact: power-of-2 scale
        in_maps.append({"xh": xh, "xs": xs, "xl": xl,
                        "eh": eh, "el": el, "emb": e})

    nc = _get_nc(TOK // P)
    LAST_RESULT = run_bass_kernel_spmd(
        nc, in_maps, core_ids=list(range(NCORES)), trace=_trace)
    results = LAST_RESULT.results

    quant = np.stack([r["quant"] for r in results])                  # [8,2048,512]
    ind = np.stack([r["ind"][:, 0].astype(np.int32) for r in results])  # [8,2048]
    dist = np.stack([r["dist"] for r in results])[None]              # [1,8,2048,8192]
    return quant, ind, dist
